# revision 13
# baseline (speedup 1.0000x reference)
"""Cross-conditional GPT2 sparse attention block on 8 Trainium2 NeuronCores.

Sharding: core = (batch b in 0..3) x (head-group g in 0..1, 6 heads each).

v4 schedule: one globally software-pipelined PE stream designed around the
TRN2 p-state rule (PE reaches 2.4 GHz only after ~3us of gap-free execution;
any stall drops it to 1.2 GHz):
  - projection chains (q/k/v/out-proj) are spread across the WHOLE kernel as
    PE filler, scheduled by dependency deadline, so the attention phase
    always has independent PE work between dependent score matmuls.
  - attv chunks of unit i-1 are interleaved with scores chunks of unit i.
  - exp PAIRING: pt slots are ordered [A0,B0,A1,B1,A2,B2,A3,B7-style] so two
    equal-width score chunks land in one 2-bank psum tile and ONE ACT exp
    covers both ([p, 2, W] APs) - halves ACT instruction count (ACT memory
    latency is ~185ns/instr of busy overhead).
  - PSUM: pairs 4 banks x2bufs, psY 2 banks (per-ich drain discipline),
    mixed pool 2 banks (projection chains + g2 tail chunks).
  - softmax denominator: ones-COLUMN at v_ones[...,0] puts the den row at
    PSUM partition 0 (reciprocal_approx_fast reads PSUM base partition 0
    directly); y rows at partitions 64..127 (PSUM APs cannot cross the
    64-partition boundary unless 64-aligned).
  - warmup dummy matmuls ramp the PE p-state while input DMAs land.
"""

import sys

sys.path.insert(0, "/opt/trn_rl_repo")

from collections import deque
from contextlib import ExitStack

import numpy as np

import concourse.bacc as bacc
import concourse.bass as bass
import concourse.mybir as mybir
import concourse.tile as tile
from concourse.bass_utils import run_bass_kernel_spmd

# ---- problem constants (hardcoded per spec) ----
B = 4
T = 512
N = 8
C = 768
NHEAD = 12
L = 3 * T + 4 * N  # 1568
P = 128
G = C // 2  # 384 channels per head-group
NH = 6  # heads per core
D = 64  # head dim
ET = C // P  # 6 e-tiles (contraction of x @ W)
CT = G // P  # 3 c-tiles of the group's channels
NJT = (L + P - 1) // P  # 13 j tiles (12x128 + 32)
SLOT = 544  # pt slot width per j-tile (max interval length)
I_CHUNKS = [(0, 512), (512, 512), (1024, 512), (1536, 32)]
ICH0 = (0, 512, 1024, 1536)
SCALE = 1.0 / 8.0  # 1/sqrt(64)
V0 = 64  # v rows base partition in psY (den/ones row at partition 0)
VW = V0 + D  # v_ones width 128: [0]=ones, [64:128]=v

F32 = mybir.dt.float32
F16 = mybir.dt.float16

_NC = None  # cached compiled Bass program


def _jl(jt):
    return P if jt < NJT - 1 else L - (NJT - 1) * P  # 128 or 32


def _ich_of(a):
    return 3 if a == 1536 else a // 512


# (group) -> per-jt score interval (a, ln).
# g0 = upper rows (i 0..512), jts 0..3; g1 = lower rows; g2 = torso+text rows.
def _grp_interval(g, jt):
    j0 = jt * P
    f0 = (jt % 4) * P if jt <= 11 else 0
    if g == 0:
        return (j0, 512 - j0) if jt <= 3 else None
    if g == 1:
        s = j0 if jt <= 3 else f0
        return (512 + s, 512 - s)
    s = j0 if jt <= 3 else f0
    return (1024 + s, 544 - s)


# diag mask kind per (group, jt in 0..11): 'T1' (tril.T) | 'T2' (strict)
def _grp_diag(g, jt):
    if g == 0:
        return "T1"
    if g == 1:
        return "T1" if jt <= 3 else "T2"
    return "T1" if jt <= 7 else "T2"


# pt slot order: A/B j-tile groups interleaved so exp pairs hit adjacent slots
_SLOT_JTS = {
    0: [0, 1, 2, 3],
    1: [0, 4, 1, 5, 2, 6, 3, 7, 8, 9, 10, 11, 12],
    2: [0, 4, 1, 5, 2, 6, 3, 7, 8, 9, 10, 11, 12],
}


def _unit_slots(g):
    """Per pt-slot: (jt, slot, a, main_cl, tail_cl)."""
    out = []
    for slot, jt in enumerate(_SLOT_JTS[g]):
        a, ln = _grp_interval(g, jt)
        out.append((jt, slot, a, min(ln, 512), max(0, ln - 512)))
    return out


def _build_program():
    nc = bacc.Bacc("TRN2", target_bir_lowering=False, debug=False)

    xT_d = nc.dram_tensor("xT", [C, L], F16, kind="ExternalInput")
    wq_d = nc.dram_tensor("wqT", [C, G], F16, kind="ExternalInput")
    wk_d = nc.dram_tensor("wkT", [C, G], F16, kind="ExternalInput")
    wv_d = nc.dram_tensor("wvT", [C, G], F16, kind="ExternalInput")
    wp_d = nc.dram_tensor("wpT", [G, C], F16, kind="ExternalInput")
    bq_d = nc.dram_tensor("bqP", [P, CT], F32, kind="ExternalInput")
    bk_d = nc.dram_tensor("bkP", [P, CT], F32, kind="ExternalInput")
    mstk_d = nc.dram_tensor("maskStk", [P, 2 * P], F16, kind="ExternalInput")
    maskt_d = nc.dram_tensor("maskTxt", [32, 1024], F16, kind="ExternalInput")
    out_d = nc.dram_tensor("out_part", [L, C], F16, kind="ExternalOutput")

    units = [(g, h) for g in range(3) for h in range(NH)]

    with tile.TileContext(nc) as tc, ExitStack() as big:
        persist = big.enter_context(tc.tile_pool(name="persist", bufs=1))
        phA = big.enter_context(tc.tile_pool(name="phA", bufs=1))
        phB = big.enter_context(tc.tile_pool(name="phB", bufs=1))
        psPair = big.enter_context(tc.tile_pool(name="psPair", bufs=2, space="PSUM"))
        psYp = big.enter_context(tc.tile_pool(name="psYp", bufs=2, space="PSUM"))
        psMix = big.enter_context(tc.tile_pool(name="psMix", bufs=2, space="PSUM"))

        # persistent SBUF tensors
        qT = persist.tile([P, CT, L], F16, name="qT")
        kT = persist.tile([P, CT, L], F16, name="kT")
        v_ones = persist.tile([P, NJT, NH, VW], F16, name="v_ones")
        maskStk = persist.tile([P, 2, P], F16, name="maskStk_sb")
        maskTx = persist.tile([32, 1024], F16, name="maskTx_sb")
        yT = persist.tile([P, CT, L], F16, name="yT")
        wp_sb = persist.tile([P, CT, C], F16, name="wp_sb")
        warm = persist.tile([P, 512], F16, name="warm")

        # small memset first: PE warmup dummies depend only on it
        nc.gpsimd.memset(warm[:], 0.125)
        # ones column at free index 0 (-> den at PSUM partition 0); cols
        # 1..63 stay 1.0 but psY rows 1..63 are never read.
        nc.gpsimd.memset(v_ones[:], 1.0)

        # ---------- input tiles + DMA, critical-path order ----------
        # First the 12 tiles the very first k/q chains need (wk, x-ich0),
        # then wq/biases/wv, then the big non-critical loads (maskStk, wp)
        # split into chunks so no queue is hogged for 30us.
        xT = phA.tile([P, ET, L], F16, name="xT_sb")
        wq_sb = phA.tile([P, ET, G], F16, name="wq_sb")
        wk_sb = phA.tile([P, ET, G], F16, name="wk_sb")
        wv_sb = phA.tile([P, ET, G], F16, name="wv_sb")
        bq_sb = phA.tile([P, CT], F32, name="bq_sb")
        bk_sb = phA.tile([P, CT], F32, name="bk_sb")

        for et in range(ET):
            nc.sync.dma_start(wk_sb[:, et, :], wk_d[et * P : (et + 1) * P, :])
            nc.sync.dma_start(xT[:, et, 0:512], xT_d[et * P : (et + 1) * P, 0:512])
        nc.sync.dma_start(bk_sb[:], bk_d[:])
        nc.sync.dma_start(bq_sb[:], bq_d[:])
        for et in range(ET):
            nc.sync.dma_start(wq_sb[:, et, :], wq_d[et * P : (et + 1) * P, :])
        for et in range(ET):
            nc.sync.dma_start(wv_sb[:, et, :], wv_d[et * P : (et + 1) * P, :])
        nc.sync.dma_start(maskStk[:], mstk_d.rearrange("p (s c) -> p s c", c=P))
        nc.sync.dma_start(maskTx[:], maskt_d[:])
        for i0, ilen in I_CHUNKS[1:]:
            for et in range(ET):
                nc.sync.dma_start(
                    xT[:, et, i0 : i0 + ilen],
                    xT_d[et * P : (et + 1) * P, i0 : i0 + ilen],
                )
        wp_v = wp_d.rearrange("(ct p) n -> p ct n", p=P)
        for ct in range(CT):
            nc.sync.dma_start(wp_sb[:, ct, :], wp_v[:, ct, :])

        # ---------- PE p-state warmup (no DMA deps) ----------
        for d in range(10):
            dt_ = psPair.tile([P, 2, 512], F32, name="ps_warm", tag="ps_pair")
            nc.tensor.matmul(
                dt_[:, d % 2, :],
                warm[:, 0:P],
                warm[:, :],
                start=True,
                stop=True,
                skip_group_check=True,
            )

        # ---------- projection chain emitters ----------
        def emit_qk_chain(dst, w_sb, b_sb, ct, ich):
            i0, ilen = I_CHUNKS[ich]
            ps = psMix.tile([P, 512], F32, name="ps_p", tag="ps_mix")
            for et in range(ET):
                nc.tensor.matmul(
                    ps[:, :ilen],
                    w_sb[:, et, ct * P : (ct + 1) * P],
                    xT[:, et, i0 : i0 + ilen],
                    start=(et == 0),
                    stop=(et == ET - 1),
                    skip_group_check=True,
                )
            nc.vector.tensor_scalar(
                dst[:, ct, i0 : i0 + ilen],
                ps[:, :ilen],
                b_sb[:, ct : ct + 1],
                None,
                mybir.AluOpType.add,
            )

        def emit_v_chain(it):
            il = _jl(it)
            ps = psMix.tile([P, 512], F32, name="ps_pv", tag="ps_mix")
            for et in range(ET):
                nc.tensor.matmul(
                    ps[:il, :G],
                    xT[:, et, it * P : it * P + il],
                    wv_sb[:, et, :],
                    start=(et == 0),
                    stop=(et == ET - 1),
                    skip_group_check=True,
                )
            nc.vector.tensor_copy(
                v_ones[:il, it, :, V0 : V0 + D],
                ps[:il, :G].rearrange("p (h d) -> p h d", h=NH),
            )

        def emit_outproj_chain(it, nch, tail=False):
            il = _jl(it)
            ps_o = psMix.tile([P, 512], F32, name="ps_po", tag="ps_mix")
            for kt in range(CT):
                nc.tensor.matmul(
                    ps_o[:il, :G],
                    yT[:, kt, it * P : it * P + il],
                    wp_sb[:, kt, nch * G : (nch + 1) * G],
                    start=(kt == 0),
                    stop=(kt == CT - 1),
                    skip_group_check=True,
                )
            o_sb = phB.tile([P, G], F16, name="o_sb", tag="o_sb", bufs=6)
            if tail and (it + nch) % 2 == 0:
                # ACT is idle in the drain tail - halve the cast serialization
                nc.scalar.copy(o_sb[:il, :], ps_o[:il, :G])
            else:
                nc.vector.tensor_copy(o_sb[:il, :], ps_o[:il, :G])
            r0 = it * P
            c0 = nch * G
            if tail and il == P:
                # split across queues + issue from the idle ACT sequencer so
                # the sync queue can run its end-of-program drain
                nc.scalar.dma_start(out_d[r0 : r0 + 64, c0 : c0 + G], o_sb[0:64, :])
                nc.scalar.dma_start(out_d[r0 + 64 : r0 + il, c0 : c0 + G],
                                    o_sb[64:il, :])
            else:
                nc.sync.dma_start(out_d[r0 : r0 + il, c0 : c0 + G], o_sb[:il, :])

        # ---------- attention emitters ----------
        urec = [dict() for _ in units]

        def kq(i, jt, ca, cl):
            g, h = units[i]
            pof = D * (h % 2)
            ct = h // 2
            jl = _jl(jt)
            return (
                kT[pof : pof + D, ct, jt * P : jt * P + jl],
                qT[pof : pof + D, ct, ca : ca + cl],
            )

        def emit_pair(i, s1, s2):
            # two equal-ish chunks -> one 2-bank psum tile -> ONE exp
            jt1, slot1, a1, cl1, _ = s1
            jt2, slot2, a2, cl2, _ = s2
            pt = urec[i]["pt"]
            tile_ = psPair.tile([P, 2, 512], F32, name="ps_s2", tag="ps_pair")
            for idx, (jt, ca, cl) in enumerate(((jt1, a1, cl1), (jt2, a2, cl2))):
                k_ap, q_ap = kq(i, jt, ca, cl)
                nc.tensor.matmul(
                    tile_[: _jl(jt), idx, :cl], k_ap, q_ap,
                    start=True, stop=True, skip_group_check=True,
                )
            W = max(cl1, cl2)
            nc.scalar.activation(
                pt[:P, slot1 : slot1 + 2, 0:W],
                tile_[:P, :, 0:W],
                mybir.ActivationFunctionType.Exp,
                bias=0.0,
                scale=SCALE,
            )

        def emit_single12(i, s):
            jt, slot, a, mcl, tcl = s
            pt = urec[i]["pt"]
            jl = _jl(jt)  # 32
            tile_ = psPair.tile([P, 2, 512], F32, name="ps_s1", tag="ps_pair")
            k_ap, q_ap = kq(i, jt, a, mcl)
            nc.tensor.matmul(
                tile_[:jl, 0, :mcl], k_ap, q_ap,
                start=True, stop=True, skip_group_check=True,
            )
            if tcl:
                k_ap, q_ap = kq(i, jt, a + 512, tcl)
                nc.tensor.matmul(
                    tile_[:jl, 1, :tcl], k_ap, q_ap,
                    start=True, stop=True, skip_group_check=True,
                )
            flat = tile_[:jl].rearrange("p a b -> p (a b)")
            nc.scalar.activation(
                pt[:jl, slot, 0 : mcl + tcl],
                flat[:, 0 : mcl + tcl],
                mybir.ActivationFunctionType.Exp,
                bias=0.0,
                scale=SCALE,
            )

        def emit_tailpair(i, s1, s2):
            # the 32-col score tails of two adjacent 544-wide slots
            pt = urec[i]["pt"]
            mix = psMix.tile([P, 512], F32, name="ps_st", tag="ps_mix")
            for idx, (jt, slot, a, mcl, tcl) in enumerate((s1, s2)):
                k_ap, q_ap = kq(i, jt, a + 512, tcl)
                nc.tensor.matmul(
                    mix[: _jl(jt), idx * 32 : idx * 32 + 32], k_ap, q_ap,
                    start=True, stop=True, skip_group_check=True,
                )
            nc.scalar.activation(
                pt[:P, s1[1] : s1[1] + 2, 512:544],
                mix[:P, 0:64].rearrange("p (a b) -> p a b", b=32),
                mybir.ActivationFunctionType.Exp,
                bias=0.0,
                scale=SCALE,
            )

        def emit_tail8(i, s):
            jt, slot, a, mcl, tcl = s
            pt = urec[i]["pt"]
            mix = psMix.tile([P, 512], F32, name="ps_st8", tag="ps_mix")
            k_ap, q_ap = kq(i, jt, a + 512, tcl)
            nc.tensor.matmul(
                mix[: _jl(jt), 0:32], k_ap, q_ap,
                start=True, stop=True, skip_group_check=True,
            )
            nc.scalar.activation(
                pt[:P, slot, 512:544],
                mix[:P, 0:32],
                mybir.ActivationFunctionType.Exp,
                bias=0.0,
                scale=SCALE,
            )

        def emit_mask_window(i, w):
            # maskStk holds just the two distinct diag masks (T1 at 0, T2 at
            # 1); window patterns are XXXX or XYXY -> broadcast APs.
            # Uniform (XXXX) windows run on the mostly-idle GpSimd to take
            # pressure off DVE in the g2 phase (GpSimd cannot touch PSUM,
            # but pt and the masks are SBUF).
            g, h = units[i]
            pt = urec[i]["pt"]
            s0 = 4 * w
            kinds = [_grp_diag(g, jt) for jt in _SLOT_JTS[g][s0 : s0 + 4]]
            dst = pt[:, s0 : s0 + 4, 0:P]
            if kinds[0] != kinds[1]:  # (T1,T2,T1,T2)
                dv = dst.rearrange("p (r t) c -> p r t c", t=2)
                mv = maskStk[:, None, 0:2, :].to_broadcast([P, 2, 2, P])
                nc.vector.tensor_tensor(dv, dv, mv, mybir.AluOpType.mult)
            else:
                idx = 0 if kinds[0] == "T1" else 1
                mv = maskStk[:, idx : idx + 1, :].to_broadcast([P, 4, P])
                nc.gpsimd.tensor_tensor(dst, dst, mv, mybir.AluOpType.mult)

        def emit_text_mask(i):
            g, h = units[i]
            pt = urec[i]["pt"]
            m0 = 0 if g == 1 else 512
            nc.vector.tensor_tensor(
                pt[0:32, 12, 0:512],
                pt[0:32, 12, 0:512],
                maskTx[0:32, m0 : m0 + 512],
                mybir.AluOpType.mult,
            )

        def attv_items(i):
            """Flat list: ("part", ich, jt, slot, pl, poff, off, first, stop)
            and ("drain", ich, is_final) items, ich groups in drain order
            (g2: ich3 first - its psY bank frees mid-block)."""
            g, h = units[i]
            groups = {}
            for jt, slot, ca, cl, soff in urec[i]["chunks"]:
                subs = [(ca, cl, soff)]
                if ca < 1536 < ca + cl:
                    subs = [
                        (ca, 1536 - ca, soff),
                        (1536, ca + cl - 1536, soff + 1536 - ca),
                    ]
                for pa, pl, poff in subs:
                    ich = _ich_of(pa)
                    groups.setdefault(ich, []).append(
                        (jt, slot, pa, pl, poff)
                    )
            ich_order = sorted(groups, key=lambda c: len(groups[c]) * 1000 + c)
            if units[i][0] == 2:
                ich_order = [3, 2]
            items = []
            for gi, ich in enumerate(ich_order):
                parts = groups[ich]
                for pi, (jt, slot, pa, pl, poff) in enumerate(parts):
                    items.append(
                        ("part", ich, jt, slot, pa, pl, poff,
                         pi == 0, pi == len(parts) - 1)
                    )
                items.append(("drain", ich, gi == len(ich_order) - 1))
            return items

        def emit_attv_item(i, item):
            g, h = units[i]
            if item[0] == "part":
                _, ich, jt, slot, pa, pl, poff, first, stop = item
                jl = _jl(jt)
                ysd = urec[i].setdefault("ps_y", {})
                if ich not in ysd:
                    ysd[ich] = psYp.tile([VW, 512], F32, name="ps_y", tag="ps_y")
                off = pa - ICH0[ich]
                nc.tensor.matmul(
                    ysd[ich][:, off : off + pl],
                    v_ones[:jl, jt, h, :],
                    urec[i]["pt"][:jl, slot, poff : poff + pl],
                    start=first,
                    stop=stop,
                    skip_group_check=True,
                )
            else:
                _, ich, is_final = item
                psy = urec[i]["ps_y"][ich]
                ilen = I_CHUNKS[ich][1]
                rc = phB.tile([1, 512], F32, name="rc", tag="rc", bufs=4)
                nc.vector.reciprocal_approx_fast(
                    out=rc[0:1, :ilen], in_=psy[0:1, :ilen]
                )
                rc_bc = phB.tile([D, 512], F32, name="rc_bc", tag="rc_bc", bufs=4)
                nc.gpsimd.partition_broadcast(rc_bc[:, :ilen], rc[0:1, :ilen])
                if is_final:
                    urec[i]["pending"] = (ich, psy, rc_bc)
                else:
                    emit_norm(i, ich, psy, rc_bc)

        def emit_norm(i, ich, psy, rc_bc):
            g, h = units[i]
            pof = D * (h % 2)
            ct = h // 2
            i0, ilen = I_CHUNKS[ich]
            nc.vector.tensor_tensor(
                yT[pof : pof + D, ct, i0 : i0 + ilen],
                psy[V0 : V0 + D, :ilen],
                rc_bc[:, :ilen],
                mybir.AluOpType.mult,
            )

        def emit_pending_norm(i):
            if "pending" in urec[i]:
                ich, psy, rc_bc = urec[i].pop("pending")
                emit_norm(i, ich, psy, rc_bc)

        # ---------- filler segments ----------
        f1 = deque()
        for ct in range(CT):
            if ct == 0:
                f1.append(lambda: emit_qk_chain(qT, wq_sb, bq_sb, 0, 1))
                for ich in (1, 2, 3):
                    f1.append(
                        lambda ich=ich: emit_qk_chain(kT, wk_sb, bk_sb, 0, ich)
                    )
                for it in range(4, NJT):
                    f1.append(lambda it=it: emit_v_chain(it))
            else:
                f1.append(lambda ct=ct: emit_qk_chain(qT, wq_sb, bq_sb, ct, 1))
                for ich in (1, 2, 3):
                    f1.append(
                        lambda ct=ct, ich=ich: emit_qk_chain(
                            kT, wk_sb, bk_sb, ct, ich
                        )
                    )
        f2 = deque()
        for ich in (2, 3):
            for ct in range(CT):
                f2.append(
                    lambda ct=ct, ich=ich: emit_qk_chain(qT, wq_sb, bq_sb, ct, ich)
                )
        f3 = deque(
            (lambda it=it, nch=nch: emit_outproj_chain(it, nch))
            for it in range(0, 4)
            for nch in range(2)
        )
        f4 = deque(
            (lambda it=it, nch=nch: emit_outproj_chain(it, nch))
            for it in range(4, 8)
            for nch in range(2)
        )
        segments = [(0, f1), (6, f2), (7, f3), (13, f4)]

        def pop_filler(block):
            for rel, dq in segments:
                if rel <= block and dq:
                    dq.popleft()()
                    return True
            return False

        QUOTA = [4, 4, 4, 3, 3, 3, 2, 2, 2, 2, 2, 2, 2, 2, 2, 2, 2, 2]

        # ---------- upfront: phase A for g0's needs ----------
        for ct in range(CT):
            emit_qk_chain(kT, wk_sb, bk_sb, ct, 0)
            emit_qk_chain(qT, wq_sb, bq_sb, ct, 0)
        for it in range(4):
            emit_v_chain(it)

        # ---------- main software-pipelined loop ----------
        for i, (g, h) in enumerate(units):
            if i == 6:
                while f1:
                    f1.popleft()()
            if i == 12:
                while f2:
                    f2.popleft()()
            if i >= 2:
                emit_pending_norm(i - 2)
            urec[i]["pt"] = phB.tile(
                [P, NJT, SLOT], F16, name="pt", tag="pt", bufs=3
            )
            slots = _unit_slots(g)
            chunks = []
            for jt, slot, a, mcl, tcl in slots:
                chunks.append((jt, slot, a, mcl, 0))
                if tcl:
                    chunks.append((jt, slot, a + 512, tcl, 512))
            urec[i]["chunks"] = chunks

            # exp-units: (fn, main_slot_done)
            eus = []
            if g == 0:
                eus.append((lambda s=slots: emit_pair(i, s[0], s[1]), 1))
                eus.append((lambda s=slots: emit_pair(i, s[2], s[3]), 3))
            else:
                for k in range(6):
                    eus.append(
                        (lambda s=slots, k=k: emit_pair(i, s[2 * k], s[2 * k + 1]),
                         2 * k + 1)
                    )
                eus.append((lambda s=slots: emit_single12(i, s[12]), 12))
                if g == 2:
                    eus.append((lambda s=slots: emit_tailpair(i, s[0], s[1]), -1))
                    eus.append((lambda s=slots: emit_tail8(i, s[8]), -1))

            av = attv_items(i - 1) if i >= 1 else []
            nE = len(eus)
            nwin = 1 if g == 0 else 3
            quota = QUOTA[i]
            pops = 0
            ai = 0
            next_w = 0
            for e_idx, (fn, sdone) in enumerate(eus):
                fn()
                while next_w < nwin and 4 * next_w + 3 <= sdone:
                    emit_mask_window(i, next_w)
                    next_w += 1
                if sdone == 12 and g >= 1:
                    emit_text_mask(i)
                tgt = (e_idx + 1) * len(av) // nE
                while ai < tgt:
                    emit_attv_item(i - 1, av[ai])
                    ai += 1
                if pops < quota and (e_idx + 1) * quota >= (pops + 1) * nE:
                    if pop_filler(i):
                        pops += 1
            while ai < len(av):
                emit_attv_item(i - 1, av[ai])
                ai += 1

        # ---------- tail ----------
        nu = len(units)
        emit_pending_norm(nu - 2)
        av = attv_items(nu - 1)
        for item in av:
            emit_attv_item(nu - 1, item)
            if item[0] == "drain" and item[1] == 3:
                # it12 out-proj needs only the ich3 norms (text rows), which
                # just completed - run it while the ich2 attv still streams
                for nch in range(2):
                    emit_outproj_chain(12, nch, tail=True)
        emit_pending_norm(nu - 1)
        for it in range(8, 12):
            for nch in range(2):
                emit_outproj_chain(it, nch, tail=True)

    nc.compile()
    return nc


def _build_mask_np(seg_starts, seg_ends):
    """True = masked. Mirrors reference._build_mask in numpy."""
    ML = 3 * T
    tril = np.tril(np.ones((T, T), dtype=bool))
    sl = np.tril(np.ones((T, T), dtype=bool), -1)
    m = np.zeros((L, L), dtype=bool)
    m[:ML, :ML] = True
    m[0:T, 0:T] = ~tril
    m[T : 2 * T, 0:T] = ~tril
    m[T : 2 * T, T : 2 * T] = ~sl
    m[T : 2 * T, 2 * T : 3 * T] = ~sl
    m[2 * T : 3 * T, 0:T] = ~tril
    m[2 * T : 3 * T, T : 2 * T] = ~tril
    m[2 * T : 3 * T, 2 * T : 3 * T] = ~sl
    m[:ML, ML:] = True
    frames = np.arange(T)[None, :, None]
    allowed = (frames >= seg_starts[:, None, :]) & (frames < seg_ends[:, None, :])
    mask = np.broadcast_to(m[None], (B, L, L)).copy()
    for row0, col_blocks in ((T, (0, 2, 3)), (2 * T, (1, 2, 3))):
        for j in col_blocks:
            c0 = ML + j * N
            mask[:, row0 : row0 + T, c0 : c0 + N] &= ~allowed
    return mask


def get_nc():
    global _NC
    if _NC is None:
        _NC = _build_program()
    return _NC


def _build_maskstk():
    r = np.arange(P)
    t1 = (r[:, None] <= r[None, :]).astype(np.float16)  # tril.T
    t2 = (r[:, None] < r[None, :]).astype(np.float16)  # strict
    stk = np.empty((P, 2, P), dtype=np.float16)
    stk[:, 0, :] = t1
    stk[:, 1, :] = t2
    return stk.reshape(P, 2 * P)


def make_in_maps(x, Wq, bq, Wk, bk, Wv, bv, Wp, bp, seg_starts, seg_ends):
    mask = _build_mask_np(np.asarray(seg_starts), np.asarray(seg_ends))
    maskstk = _build_maskstk()
    in_maps = []
    for core in range(8):
        b, g = core // 2, core % 2
        gs = slice(g * G, (g + 1) * G)
        allowT = ~mask[b].T  # [j, i]
        maskTx = np.ascontiguousarray(
            allowT[1536:1568, 512:1536].astype(np.float16)
        )
        in_maps.append(
            {
                "xT": np.ascontiguousarray(x[b].T).astype(np.float16),
                "wqT": np.ascontiguousarray(Wq[gs, :].T).astype(np.float16),
                "wkT": np.ascontiguousarray(Wk[gs, :].T).astype(np.float16),
                "wvT": np.ascontiguousarray(Wv[gs, :].T).astype(np.float16),
                "wpT": np.ascontiguousarray(Wp[:, gs].T).astype(np.float16),
                "bqP": np.ascontiguousarray(bq[gs].reshape(CT, P).T),
                "bkP": np.ascontiguousarray(bk[gs].reshape(CT, P).T),
                "maskStk": maskstk,
                "maskTxt": maskTx,
            }
        )
    return in_maps


def kernel(x, Wq, bq, Wk, bk, Wv, bv, Wp, bp, seg_starts, seg_ends, T_motion=None,
           N=None, _trace=False, **_unused):
    x = np.asarray(x, np.float32)
    args = [np.asarray(a, np.float32) for a in (Wq, bq, Wk, bk, Wv, bv, Wp, bp)]
    Wq, bq, Wk, bk, Wv, bv, Wp, bp = args
    nc = get_nc()
    in_maps = make_in_maps(x, Wq, bq, Wk, bk, Wv, bv, Wp, bp, seg_starts, seg_ends)
    res = run_bass_kernel_spmd(nc, in_maps, core_ids=list(range(8)), trace=_trace)
    parts = [np.asarray(r["out_part"], np.float32) for r in res.results]
    # v-bias folds into the output bias exactly: att rows sum to 1, so
    # y = att@(v+bv) = att@v + bv, and (y+bv)@Wp.T = y@Wp.T + bv@Wp.T
    bp_eff = bp + bv @ Wp.T
    y = np.empty((B, L, C), np.float32)
    for b in range(B):
        y[b] = parts[2 * b] + parts[2 * b + 1] + bp_eff
    if _trace:
        kernel.last_results = res
    return y


# revision 14
# speedup vs baseline: 2.1082x; 2.1082x over previous
"""Cross-conditional GPT2 sparse attention block on 8 Trainium2 NeuronCores.

Sharding: core = (batch b in 0..3) x (head-group g in 0..1, 6 heads each).

v4 schedule: one globally software-pipelined PE stream designed around the
TRN2 p-state rule (PE reaches 2.4 GHz only after ~3us of gap-free execution;
any stall drops it to 1.2 GHz):
  - projection chains (q/k/v/out-proj) are spread across the WHOLE kernel as
    PE filler, scheduled by dependency deadline, so the attention phase
    always has independent PE work between dependent score matmuls.
  - attv chunks of unit i-1 are interleaved with scores chunks of unit i.
  - exp PAIRING: pt slots are ordered [A0,B0,A1,B1,A2,B2,A3,B7-style] so two
    equal-width score chunks land in one 2-bank psum tile and ONE ACT exp
    covers both ([p, 2, W] APs) - halves ACT instruction count (ACT memory
    latency is ~185ns/instr of busy overhead).
  - PSUM: pairs 4 banks x2bufs, psY 2 banks (per-ich drain discipline),
    mixed pool 2 banks (projection chains + g2 tail chunks).
  - softmax denominator: ones-COLUMN at v_ones[...,0] puts the den row at
    PSUM partition 0 (reciprocal_approx_fast reads PSUM base partition 0
    directly); y rows at partitions 64..127 (PSUM APs cannot cross the
    64-partition boundary unless 64-aligned).
  - warmup dummy matmuls ramp the PE p-state while input DMAs land.
"""

import sys

sys.path.insert(0, "/opt/trn_rl_repo")

from collections import deque
from contextlib import ExitStack

import numpy as np

import concourse.bacc as bacc
import concourse.bass as bass
import concourse.mybir as mybir
import concourse.tile as tile
from concourse.bass_utils import run_bass_kernel_spmd

# ---- problem constants (hardcoded per spec) ----
B = 4
T = 512
N = 8
C = 768
NHEAD = 12
L = 3 * T + 4 * N  # 1568
P = 128
G = C // 2  # 384 channels per head-group
NH = 6  # heads per core
D = 64  # head dim
ET = C // P  # 6 e-tiles (contraction of x @ W)
CT = G // P  # 3 c-tiles of the group's channels
NJT = (L + P - 1) // P  # 13 j tiles (12x128 + 32)
SLOT = 544  # pt slot width per j-tile (max interval length)
I_CHUNKS = [(0, 512), (512, 512), (1024, 512), (1536, 32)]
ICH0 = (0, 512, 1024, 1536)
SCALE = 1.0 / 8.0  # 1/sqrt(64)
V0 = 64  # v rows base partition in psY (den/ones row at partition 0)
VW = V0 + D  # v_ones width 128: [0]=ones, [64:128]=v

F32 = mybir.dt.float32
F16 = mybir.dt.float16

_NC = None  # cached compiled Bass program


def _jl(jt):
    return P if jt < NJT - 1 else L - (NJT - 1) * P  # 128 or 32


def _ich_of(a):
    return 3 if a == 1536 else a // 512


# (group) -> per-jt score interval (a, ln).
# g0 = upper rows (i 0..512), jts 0..3; g1 = lower rows; g2 = torso+text rows.
def _grp_interval(g, jt):
    j0 = jt * P
    f0 = (jt % 4) * P if jt <= 11 else 0
    if g == 0:
        return (j0, 512 - j0) if jt <= 3 else None
    if g == 1:
        s = j0 if jt <= 3 else f0
        return (512 + s, 512 - s)
    s = j0 if jt <= 3 else f0
    return (1024 + s, 544 - s)


# diag mask kind per (group, jt in 0..11): 'T1' (tril.T) | 'T2' (strict)
def _grp_diag(g, jt):
    if g == 0:
        return "T1"
    if g == 1:
        return "T1" if jt <= 3 else "T2"
    return "T1" if jt <= 7 else "T2"


# pt slot order: A/B j-tile groups interleaved so exp pairs hit adjacent slots
_SLOT_JTS = {
    0: [0, 1, 2, 3],
    1: [0, 4, 1, 5, 2, 6, 3, 7, 8, 9, 10, 11, 12],
    2: [0, 4, 1, 5, 2, 6, 3, 7, 8, 9, 10, 11, 12],
}


def _unit_slots(g):
    """Per pt-slot: (jt, slot, a, main_cl, tail_cl)."""
    out = []
    for slot, jt in enumerate(_SLOT_JTS[g]):
        a, ln = _grp_interval(g, jt)
        out.append((jt, slot, a, min(ln, 512), max(0, ln - 512)))
    return out


def _build_program():
    nc = bacc.Bacc("TRN2", target_bir_lowering=False, debug=False)

    xT_d = nc.dram_tensor("xT", [C, L], F16, kind="ExternalInput")
    wq_d = nc.dram_tensor("wqT", [C, G], F16, kind="ExternalInput")
    wk_d = nc.dram_tensor("wkT", [C, G], F16, kind="ExternalInput")
    wv_d = nc.dram_tensor("wvT", [C, G], F16, kind="ExternalInput")
    wp_d = nc.dram_tensor("wpT", [G, C], F16, kind="ExternalInput")
    bq_d = nc.dram_tensor("bqP", [P, CT], F32, kind="ExternalInput")
    bk_d = nc.dram_tensor("bkP", [P, CT], F32, kind="ExternalInput")
    mstk_d = nc.dram_tensor("maskStk", [P, 2 * P], F16, kind="ExternalInput")
    maskt_d = nc.dram_tensor("maskTxt", [32, 1024], F16, kind="ExternalInput")
    out_d = nc.dram_tensor("out_part", [L, C], F16, kind="ExternalOutput")

    units = [(g, h) for g in range(3) for h in range(NH)]

    with tile.TileContext(nc) as tc, ExitStack() as big:
        persist = big.enter_context(tc.tile_pool(name="persist", bufs=1))
        phA = big.enter_context(tc.tile_pool(name="phA", bufs=1))
        phB = big.enter_context(tc.tile_pool(name="phB", bufs=1))
        psPair = big.enter_context(tc.tile_pool(name="psPair", bufs=2, space="PSUM"))
        psYp = big.enter_context(tc.tile_pool(name="psYp", bufs=2, space="PSUM"))
        psMix = big.enter_context(tc.tile_pool(name="psMix", bufs=2, space="PSUM"))

        # persistent SBUF tensors
        qT = persist.tile([P, CT, L], F16, name="qT")
        kT = persist.tile([P, CT, L], F16, name="kT")
        v_ones = persist.tile([P, NJT, NH, VW], F16, name="v_ones")
        maskStk = persist.tile([P, 2, P], F16, name="maskStk_sb")
        maskTx = persist.tile([32, 1024], F16, name="maskTx_sb")
        yT = persist.tile([P, CT, L], F16, name="yT")
        wp_sb = persist.tile([P, CT, C], F16, name="wp_sb")
        warm = persist.tile([P, 512], F16, name="warm")

        # small memset first: PE warmup dummies depend only on it
        nc.gpsimd.memset(warm[:], 0.125)
        # ones column at free index 0 (-> den at PSUM partition 0); cols
        # 1..63 stay 1.0 but psY rows 1..63 are never read.
        nc.gpsimd.memset(v_ones[:], 1.0)

        # ---------- input tiles + DMA, critical-path order ----------
        # First the 12 tiles the very first k/q chains need (wk, x-ich0),
        # then wq/biases/wv, then the big non-critical loads (maskStk, wp)
        # split into chunks so no queue is hogged for 30us.
        xT = phA.tile([P, ET, L], F16, name="xT_sb")
        wq_sb = phA.tile([P, ET, G], F16, name="wq_sb")
        wk_sb = phA.tile([P, ET, G], F16, name="wk_sb")
        wv_sb = phA.tile([P, ET, G], F16, name="wv_sb")
        bq_sb = phA.tile([P, CT], F32, name="bq_sb")
        bk_sb = phA.tile([P, CT], F32, name="bk_sb")

        for et in range(ET):
            nc.sync.dma_start(wk_sb[:, et, :], wk_d[et * P : (et + 1) * P, :])
            nc.sync.dma_start(xT[:, et, 0:512], xT_d[et * P : (et + 1) * P, 0:512])
        nc.sync.dma_start(bk_sb[:], bk_d[:])
        nc.sync.dma_start(bq_sb[:], bq_d[:])
        for et in range(ET):
            nc.sync.dma_start(wq_sb[:, et, :], wq_d[et * P : (et + 1) * P, :])
        for et in range(ET):
            nc.sync.dma_start(wv_sb[:, et, :], wv_d[et * P : (et + 1) * P, :])
        nc.sync.dma_start(maskStk[:], mstk_d.rearrange("p (s c) -> p s c", c=P))
        nc.sync.dma_start(maskTx[:], maskt_d[:])
        for i0, ilen in I_CHUNKS[1:]:
            for et in range(ET):
                nc.sync.dma_start(
                    xT[:, et, i0 : i0 + ilen],
                    xT_d[et * P : (et + 1) * P, i0 : i0 + ilen],
                )
        wp_v = wp_d.rearrange("(ct p) n -> p ct n", p=P)
        for ct in range(CT):
            nc.sync.dma_start(wp_sb[:, ct, :], wp_v[:, ct, :])

        # ---------- PE p-state warmup (no DMA deps) ----------
        for d in range(10):
            dt_ = psPair.tile([P, 2, 512], F32, name="ps_warm", tag="ps_pair")
            nc.tensor.matmul(
                dt_[:, d % 2, :],
                warm[:, 0:P],
                warm[:, :],
                start=True,
                stop=True,
                skip_group_check=True,
            )

        # ---------- projection chain emitters ----------
        def emit_qk_chain(dst, w_sb, b_sb, ct, ich):
            i0, ilen = I_CHUNKS[ich]
            ps = psMix.tile([P, 512], F32, name="ps_p", tag="ps_mix")
            for et in range(ET):
                nc.tensor.matmul(
                    ps[:, :ilen],
                    w_sb[:, et, ct * P : (ct + 1) * P],
                    xT[:, et, i0 : i0 + ilen],
                    start=(et == 0),
                    stop=(et == ET - 1),
                    skip_group_check=True,
                )
            nc.vector.tensor_scalar(
                dst[:, ct, i0 : i0 + ilen],
                ps[:, :ilen],
                b_sb[:, ct : ct + 1],
                None,
                mybir.AluOpType.add,
            )

        def emit_v_chain(it):
            il = _jl(it)
            ps = psMix.tile([P, 512], F32, name="ps_pv", tag="ps_mix")
            for et in range(ET):
                nc.tensor.matmul(
                    ps[:il, :G],
                    xT[:, et, it * P : it * P + il],
                    wv_sb[:, et, :],
                    start=(et == 0),
                    stop=(et == ET - 1),
                    skip_group_check=True,
                )
            nc.vector.tensor_copy(
                v_ones[:il, it, :, V0 : V0 + D],
                ps[:il, :G].rearrange("p (h d) -> p h d", h=NH),
            )

        def emit_outproj_chain(it, nch, tail=False):
            il = _jl(it)
            ps_o = psMix.tile([P, 512], F32, name="ps_po", tag="ps_mix")
            for kt in range(CT):
                nc.tensor.matmul(
                    ps_o[:il, :G],
                    yT[:, kt, it * P : it * P + il],
                    wp_sb[:, kt, nch * G : (nch + 1) * G],
                    start=(kt == 0),
                    stop=(kt == CT - 1),
                    skip_group_check=True,
                )
            o_sb = phB.tile([P, G], F16, name="o_sb", tag="o_sb", bufs=6)
            if tail and (it + nch) % 2 == 0:
                # ACT is idle in the drain tail - halve the cast serialization
                nc.scalar.copy(o_sb[:il, :], ps_o[:il, :G])
            else:
                nc.vector.tensor_copy(o_sb[:il, :], ps_o[:il, :G])
            r0 = it * P
            c0 = nch * G
            if tail and il == P:
                # split across queues + issue from the idle ACT sequencer so
                # the sync queue can run its end-of-program drain
                nc.scalar.dma_start(out_d[r0 : r0 + 64, c0 : c0 + G], o_sb[0:64, :])
                nc.scalar.dma_start(out_d[r0 + 64 : r0 + il, c0 : c0 + G],
                                    o_sb[64:il, :])
            else:
                nc.sync.dma_start(out_d[r0 : r0 + il, c0 : c0 + G], o_sb[:il, :])

        # ---------- attention emitters ----------
        urec = [dict() for _ in units]

        def kq(i, jt, ca, cl):
            g, h = units[i]
            pof = D * (h % 2)
            ct = h // 2
            jl = _jl(jt)
            return (
                kT[pof : pof + D, ct, jt * P : jt * P + jl],
                qT[pof : pof + D, ct, ca : ca + cl],
            )

        def emit_pair(i, s1, s2):
            # two equal-ish chunks -> one 2-bank psum tile -> ONE exp
            jt1, slot1, a1, cl1, _ = s1
            jt2, slot2, a2, cl2, _ = s2
            pt = urec[i]["pt"]
            tile_ = psPair.tile([P, 2, 512], F32, name="ps_s2", tag="ps_pair")
            for idx, (jt, ca, cl) in enumerate(((jt1, a1, cl1), (jt2, a2, cl2))):
                k_ap, q_ap = kq(i, jt, ca, cl)
                nc.tensor.matmul(
                    tile_[: _jl(jt), idx, :cl], k_ap, q_ap,
                    start=True, stop=True, skip_group_check=True,
                )
            W = max(cl1, cl2)
            nc.scalar.activation(
                pt[:P, slot1 : slot1 + 2, 0:W],
                tile_[:P, :, 0:W],
                mybir.ActivationFunctionType.Exp,
                bias=0.0,
                scale=SCALE,
            )

        def emit_single12(i, s):
            jt, slot, a, mcl, tcl = s
            pt = urec[i]["pt"]
            jl = _jl(jt)  # 32
            tile_ = psPair.tile([P, 2, 512], F32, name="ps_s1", tag="ps_pair")
            k_ap, q_ap = kq(i, jt, a, mcl)
            nc.tensor.matmul(
                tile_[:jl, 0, :mcl], k_ap, q_ap,
                start=True, stop=True, skip_group_check=True,
            )
            if tcl:
                k_ap, q_ap = kq(i, jt, a + 512, tcl)
                nc.tensor.matmul(
                    tile_[:jl, 1, :tcl], k_ap, q_ap,
                    start=True, stop=True, skip_group_check=True,
                )
            flat = tile_[:jl].rearrange("p a b -> p (a b)")
            nc.scalar.activation(
                pt[:jl, slot, 0 : mcl + tcl],
                flat[:, 0 : mcl + tcl],
                mybir.ActivationFunctionType.Exp,
                bias=0.0,
                scale=SCALE,
            )

        def emit_tailpair(i, s1, s2):
            # the 32-col score tails of two adjacent 544-wide slots
            pt = urec[i]["pt"]
            mix = psMix.tile([P, 512], F32, name="ps_st", tag="ps_mix")
            for idx, (jt, slot, a, mcl, tcl) in enumerate((s1, s2)):
                k_ap, q_ap = kq(i, jt, a + 512, tcl)
                nc.tensor.matmul(
                    mix[: _jl(jt), idx * 32 : idx * 32 + 32], k_ap, q_ap,
                    start=True, stop=True, skip_group_check=True,
                )
            nc.scalar.activation(
                pt[:P, s1[1] : s1[1] + 2, 512:544],
                mix[:P, 0:64].rearrange("p (a b) -> p a b", b=32),
                mybir.ActivationFunctionType.Exp,
                bias=0.0,
                scale=SCALE,
            )

        def emit_tail8(i, s):
            jt, slot, a, mcl, tcl = s
            pt = urec[i]["pt"]
            mix = psMix.tile([P, 512], F32, name="ps_st8", tag="ps_mix")
            k_ap, q_ap = kq(i, jt, a + 512, tcl)
            nc.tensor.matmul(
                mix[: _jl(jt), 0:32], k_ap, q_ap,
                start=True, stop=True, skip_group_check=True,
            )
            nc.scalar.activation(
                pt[:P, slot, 512:544],
                mix[:P, 0:32],
                mybir.ActivationFunctionType.Exp,
                bias=0.0,
                scale=SCALE,
            )

        def emit_mask_window(i, w):
            # maskStk holds just the two distinct diag masks (T1 at 0, T2 at
            # 1); window patterns are XXXX or XYXY -> broadcast APs.
            # (GpSimd is too slow for these: ~1.1us/window + microcode
            # library reloads against PartitionBroadcast, and the windows
            # gate attv - keep them on DVE.)
            g, h = units[i]
            pt = urec[i]["pt"]
            s0 = 4 * w
            kinds = [_grp_diag(g, jt) for jt in _SLOT_JTS[g][s0 : s0 + 4]]
            dst = pt[:, s0 : s0 + 4, 0:P]
            if kinds[0] != kinds[1]:  # (T1,T2,T1,T2)
                dv = dst.rearrange("p (r t) c -> p r t c", t=2)
                mv = maskStk[:, None, 0:2, :].to_broadcast([P, 2, 2, P])
                nc.vector.tensor_tensor(dv, dv, mv, mybir.AluOpType.mult)
            else:
                idx = 0 if kinds[0] == "T1" else 1
                mv = maskStk[:, idx : idx + 1, :].to_broadcast([P, 4, P])
                nc.vector.tensor_tensor(dst, dst, mv, mybir.AluOpType.mult)

        def emit_text_mask(i):
            g, h = units[i]
            pt = urec[i]["pt"]
            m0 = 0 if g == 1 else 512
            nc.vector.tensor_tensor(
                pt[0:32, 12, 0:512],
                pt[0:32, 12, 0:512],
                maskTx[0:32, m0 : m0 + 512],
                mybir.AluOpType.mult,
            )

        def attv_items(i):
            """Flat list: ("part", ich, jt, slot, pl, poff, off, first, stop)
            and ("drain", ich, is_final) items, ich groups in drain order
            (g2: ich3 first - its psY bank frees mid-block)."""
            g, h = units[i]
            groups = {}
            for jt, slot, ca, cl, soff in urec[i]["chunks"]:
                subs = [(ca, cl, soff)]
                if ca < 1536 < ca + cl:
                    subs = [
                        (ca, 1536 - ca, soff),
                        (1536, ca + cl - 1536, soff + 1536 - ca),
                    ]
                for pa, pl, poff in subs:
                    ich = _ich_of(pa)
                    groups.setdefault(ich, []).append(
                        (jt, slot, pa, pl, poff)
                    )
            ich_order = sorted(groups, key=lambda c: len(groups[c]) * 1000 + c)
            if units[i][0] == 2:
                ich_order = [3, 2]
            items = []
            for gi, ich in enumerate(ich_order):
                parts = groups[ich]
                for pi, (jt, slot, pa, pl, poff) in enumerate(parts):
                    items.append(
                        ("part", ich, jt, slot, pa, pl, poff,
                         pi == 0, pi == len(parts) - 1)
                    )
                items.append(("drain", ich, gi == len(ich_order) - 1))
            return items

        def emit_attv_item(i, item):
            g, h = units[i]
            if item[0] == "part":
                _, ich, jt, slot, pa, pl, poff, first, stop = item
                jl = _jl(jt)
                ysd = urec[i].setdefault("ps_y", {})
                if ich not in ysd:
                    ysd[ich] = psYp.tile([VW, 512], F32, name="ps_y", tag="ps_y")
                off = pa - ICH0[ich]
                nc.tensor.matmul(
                    ysd[ich][:, off : off + pl],
                    v_ones[:jl, jt, h, :],
                    urec[i]["pt"][:jl, slot, poff : poff + pl],
                    start=first,
                    stop=stop,
                    skip_group_check=True,
                )
            else:
                _, ich, is_final = item
                psy = urec[i]["ps_y"][ich]
                ilen = I_CHUNKS[ich][1]
                rc = phB.tile([1, 512], F32, name="rc", tag="rc", bufs=4)
                nc.vector.reciprocal_approx_fast(
                    out=rc[0:1, :ilen], in_=psy[0:1, :ilen]
                )
                rc_bc = phB.tile([D, 512], F32, name="rc_bc", tag="rc_bc", bufs=4)
                nc.gpsimd.partition_broadcast(rc_bc[:, :ilen], rc[0:1, :ilen])
                if is_final:
                    urec[i]["pending"] = (ich, psy, rc_bc)
                else:
                    emit_norm(i, ich, psy, rc_bc)

        def emit_norm(i, ich, psy, rc_bc):
            g, h = units[i]
            pof = D * (h % 2)
            ct = h // 2
            i0, ilen = I_CHUNKS[ich]
            nc.vector.tensor_tensor(
                yT[pof : pof + D, ct, i0 : i0 + ilen],
                psy[V0 : V0 + D, :ilen],
                rc_bc[:, :ilen],
                mybir.AluOpType.mult,
            )

        def emit_pending_norm(i):
            if "pending" in urec[i]:
                ich, psy, rc_bc = urec[i].pop("pending")
                emit_norm(i, ich, psy, rc_bc)

        # ---------- filler segments ----------
        f1 = deque()
        for ct in range(CT):
            if ct == 0:
                f1.append(lambda: emit_qk_chain(qT, wq_sb, bq_sb, 0, 1))
                for ich in (1, 2, 3):
                    f1.append(
                        lambda ich=ich: emit_qk_chain(kT, wk_sb, bk_sb, 0, ich)
                    )
                for it in range(4, NJT):
                    f1.append(lambda it=it: emit_v_chain(it))
            else:
                f1.append(lambda ct=ct: emit_qk_chain(qT, wq_sb, bq_sb, ct, 1))
                for ich in (1, 2, 3):
                    f1.append(
                        lambda ct=ct, ich=ich: emit_qk_chain(
                            kT, wk_sb, bk_sb, ct, ich
                        )
                    )
        f2 = deque()
        for ich in (2, 3):
            for ct in range(CT):
                f2.append(
                    lambda ct=ct, ich=ich: emit_qk_chain(qT, wq_sb, bq_sb, ct, ich)
                )
        f3 = deque(
            (lambda it=it, nch=nch: emit_outproj_chain(it, nch))
            for it in range(0, 4)
            for nch in range(2)
        )
        f4 = deque(
            (lambda it=it, nch=nch: emit_outproj_chain(it, nch))
            for it in range(4, 8)
            for nch in range(2)
        )
        segments = [(0, f1), (6, f2), (7, f3), (13, f4)]

        def pop_filler(block):
            for rel, dq in segments:
                if rel <= block and dq:
                    dq.popleft()()
                    return True
            return False

        QUOTA = [4, 4, 4, 3, 3, 3, 2, 2, 2, 2, 2, 2, 2, 2, 2, 2, 2, 2]

        # ---------- upfront: phase A for g0's needs ----------
        for ct in range(CT):
            emit_qk_chain(kT, wk_sb, bk_sb, ct, 0)
            emit_qk_chain(qT, wq_sb, bq_sb, ct, 0)
        for it in range(4):
            emit_v_chain(it)

        # ---------- main software-pipelined loop ----------
        for i, (g, h) in enumerate(units):
            if i == 6:
                while f1:
                    f1.popleft()()
            if i == 12:
                while f2:
                    f2.popleft()()
            if i >= 2:
                emit_pending_norm(i - 2)
            urec[i]["pt"] = phB.tile(
                [P, NJT, SLOT], F16, name="pt", tag="pt", bufs=3
            )
            slots = _unit_slots(g)
            chunks = []
            for jt, slot, a, mcl, tcl in slots:
                chunks.append((jt, slot, a, mcl, 0))
                if tcl:
                    chunks.append((jt, slot, a + 512, tcl, 512))
            urec[i]["chunks"] = chunks

            # exp-units: (fn, main_slot_done)
            eus = []
            if g == 0:
                eus.append((lambda s=slots: emit_pair(i, s[0], s[1]), 1))
                eus.append((lambda s=slots: emit_pair(i, s[2], s[3]), 3))
            else:
                for k in range(6):
                    eus.append(
                        (lambda s=slots, k=k: emit_pair(i, s[2 * k], s[2 * k + 1]),
                         2 * k + 1)
                    )
                eus.append((lambda s=slots: emit_single12(i, s[12]), 12))
                if g == 2:
                    eus.append((lambda s=slots: emit_tailpair(i, s[0], s[1]), -1))
                    eus.append((lambda s=slots: emit_tail8(i, s[8]), -1))

            av = attv_items(i - 1) if i >= 1 else []
            nE = len(eus)
            nwin = 1 if g == 0 else 3
            quota = QUOTA[i]
            pops = 0
            ai = 0
            next_w = 0
            for e_idx, (fn, sdone) in enumerate(eus):
                fn()
                while next_w < nwin and 4 * next_w + 3 <= sdone:
                    emit_mask_window(i, next_w)
                    next_w += 1
                if sdone == 12 and g >= 1:
                    emit_text_mask(i)
                tgt = (e_idx + 1) * len(av) // nE
                while ai < tgt:
                    emit_attv_item(i - 1, av[ai])
                    ai += 1
                if pops < quota and (e_idx + 1) * quota >= (pops + 1) * nE:
                    if pop_filler(i):
                        pops += 1
            while ai < len(av):
                emit_attv_item(i - 1, av[ai])
                ai += 1

        # ---------- tail ----------
        nu = len(units)
        emit_pending_norm(nu - 2)
        av = attv_items(nu - 1)
        for item in av:
            emit_attv_item(nu - 1, item)
            if item[0] == "drain" and item[1] == 3:
                # it12 out-proj needs only the ich3 norms (text rows), which
                # just completed - run it while the ich2 attv still streams
                for nch in range(2):
                    emit_outproj_chain(12, nch, tail=True)
        emit_pending_norm(nu - 1)
        for it in range(8, 12):
            for nch in range(2):
                emit_outproj_chain(it, nch, tail=True)

    nc.compile()
    return nc


def _build_mask_np(seg_starts, seg_ends):
    """True = masked. Mirrors reference._build_mask in numpy."""
    ML = 3 * T
    tril = np.tril(np.ones((T, T), dtype=bool))
    sl = np.tril(np.ones((T, T), dtype=bool), -1)
    m = np.zeros((L, L), dtype=bool)
    m[:ML, :ML] = True
    m[0:T, 0:T] = ~tril
    m[T : 2 * T, 0:T] = ~tril
    m[T : 2 * T, T : 2 * T] = ~sl
    m[T : 2 * T, 2 * T : 3 * T] = ~sl
    m[2 * T : 3 * T, 0:T] = ~tril
    m[2 * T : 3 * T, T : 2 * T] = ~tril
    m[2 * T : 3 * T, 2 * T : 3 * T] = ~sl
    m[:ML, ML:] = True
    frames = np.arange(T)[None, :, None]
    allowed = (frames >= seg_starts[:, None, :]) & (frames < seg_ends[:, None, :])
    mask = np.broadcast_to(m[None], (B, L, L)).copy()
    for row0, col_blocks in ((T, (0, 2, 3)), (2 * T, (1, 2, 3))):
        for j in col_blocks:
            c0 = ML + j * N
            mask[:, row0 : row0 + T, c0 : c0 + N] &= ~allowed
    return mask


def get_nc():
    global _NC
    if _NC is None:
        _NC = _build_program()
    return _NC


def _build_maskstk():
    r = np.arange(P)
    t1 = (r[:, None] <= r[None, :]).astype(np.float16)  # tril.T
    t2 = (r[:, None] < r[None, :]).astype(np.float16)  # strict
    stk = np.empty((P, 2, P), dtype=np.float16)
    stk[:, 0, :] = t1
    stk[:, 1, :] = t2
    return stk.reshape(P, 2 * P)


def make_in_maps(x, Wq, bq, Wk, bk, Wv, bv, Wp, bp, seg_starts, seg_ends):
    mask = _build_mask_np(np.asarray(seg_starts), np.asarray(seg_ends))
    maskstk = _build_maskstk()
    in_maps = []
    for core in range(8):
        b, g = core // 2, core % 2
        gs = slice(g * G, (g + 1) * G)
        allowT = ~mask[b].T  # [j, i]
        maskTx = np.ascontiguousarray(
            allowT[1536:1568, 512:1536].astype(np.float16)
        )
        in_maps.append(
            {
                "xT": np.ascontiguousarray(x[b].T).astype(np.float16),
                "wqT": np.ascontiguousarray(Wq[gs, :].T).astype(np.float16),
                "wkT": np.ascontiguousarray(Wk[gs, :].T).astype(np.float16),
                "wvT": np.ascontiguousarray(Wv[gs, :].T).astype(np.float16),
                "wpT": np.ascontiguousarray(Wp[:, gs].T).astype(np.float16),
                "bqP": np.ascontiguousarray(bq[gs].reshape(CT, P).T),
                "bkP": np.ascontiguousarray(bk[gs].reshape(CT, P).T),
                "maskStk": maskstk,
                "maskTxt": maskTx,
            }
        )
    return in_maps


def kernel(x, Wq, bq, Wk, bk, Wv, bv, Wp, bp, seg_starts, seg_ends, T_motion=None,
           N=None, _trace=False, **_unused):
    x = np.asarray(x, np.float32)
    args = [np.asarray(a, np.float32) for a in (Wq, bq, Wk, bk, Wv, bv, Wp, bp)]
    Wq, bq, Wk, bk, Wv, bv, Wp, bp = args
    nc = get_nc()
    in_maps = make_in_maps(x, Wq, bq, Wk, bk, Wv, bv, Wp, bp, seg_starts, seg_ends)
    res = run_bass_kernel_spmd(nc, in_maps, core_ids=list(range(8)), trace=_trace)
    parts = [np.asarray(r["out_part"], np.float32) for r in res.results]
    # v-bias folds into the output bias exactly: att rows sum to 1, so
    # y = att@(v+bv) = att@v + bv, and (y+bv)@Wp.T = y@Wp.T + bv@Wp.T
    bp_eff = bp + bv @ Wp.T
    y = np.empty((B, L, C), np.float32)
    for b in range(B):
        y[b] = parts[2 * b] + parts[2 * b + 1] + bp_eff
    if _trace:
        kernel.last_results = res
    return y


# revision 17
# speedup vs baseline: 2.1787x; 1.0335x over previous
"""Cross-conditional GPT2 sparse attention block on 8 Trainium2 NeuronCores.

Sharding: core = (batch b in 0..3) x (head-group g in 0..1, 6 heads each).

v4 schedule: one globally software-pipelined PE stream designed around the
TRN2 p-state rule (PE reaches 2.4 GHz only after ~3us of gap-free execution;
any stall drops it to 1.2 GHz):
  - projection chains (q/k/v/out-proj) are spread across the WHOLE kernel as
    PE filler, scheduled by dependency deadline, so the attention phase
    always has independent PE work between dependent score matmuls.
  - attv chunks of unit i-1 are interleaved with scores chunks of unit i.
  - exp PAIRING: pt slots are ordered [A0,B0,A1,B1,A2,B2,A3,B7-style] so two
    equal-width score chunks land in one 2-bank psum tile and ONE ACT exp
    covers both ([p, 2, W] APs) - halves ACT instruction count (ACT memory
    latency is ~185ns/instr of busy overhead).
  - PSUM: pairs 4 banks x2bufs, psY 2 banks (per-ich drain discipline),
    mixed pool 2 banks (projection chains + g2 tail chunks).
  - softmax denominator: ones-COLUMN at v_ones[...,0] puts the den row at
    PSUM partition 0 (reciprocal_approx_fast reads PSUM base partition 0
    directly); y rows at partitions 64..127 (PSUM APs cannot cross the
    64-partition boundary unless 64-aligned).
  - warmup dummy matmuls ramp the PE p-state while input DMAs land.
"""

import sys

sys.path.insert(0, "/opt/trn_rl_repo")

from collections import deque
from contextlib import ExitStack

import numpy as np

import concourse.bacc as bacc
import concourse.bass as bass
import concourse.mybir as mybir
import concourse.tile as tile
from concourse.bass_utils import run_bass_kernel_spmd

# ---- problem constants (hardcoded per spec) ----
B = 4
T = 512
N = 8
C = 768
NHEAD = 12
L = 3 * T + 4 * N  # 1568
P = 128
G = C // 2  # 384 channels per head-group
NH = 6  # heads per core
D = 64  # head dim
ET = C // P  # 6 e-tiles (contraction of x @ W)
CT = G // P  # 3 c-tiles of the group's channels
NJT = (L + P - 1) // P  # 13 j tiles (12x128 + 32)
SLOT = 544  # pt slot width per j-tile (max interval length)
I_CHUNKS = [(0, 512), (512, 512), (1024, 512), (1536, 32)]
ICH0 = (0, 512, 1024, 1536)
SCALE = 1.0 / 8.0  # 1/sqrt(64)
V0 = 64  # v rows base partition in psY (den/ones row at partition 0)
VW = V0 + D  # v_ones width 128: [0]=ones, [64:128]=v

F32 = mybir.dt.float32
F16 = mybir.dt.float16

_NC = None  # cached compiled Bass program


def _jl(jt):
    return P if jt < NJT - 1 else L - (NJT - 1) * P  # 128 or 32


def _ich_of(a):
    return 3 if a == 1536 else a // 512


# (group) -> per-jt score interval (a, ln).
# g0 = upper rows (i 0..512), jts 0..3; g1 = lower rows; g2 = torso+text rows.
def _grp_interval(g, jt):
    j0 = jt * P
    f0 = (jt % 4) * P if jt <= 11 else 0
    if g == 0:
        return (j0, 512 - j0) if jt <= 3 else None
    if g == 1:
        s = j0 if jt <= 3 else f0
        return (512 + s, 512 - s)
    s = j0 if jt <= 3 else f0
    return (1024 + s, 544 - s)


# diag mask kind per (group, jt in 0..11): 'T1' (tril.T) | 'T2' (strict)
def _grp_diag(g, jt):
    if g == 0:
        return "T1"
    if g == 1:
        return "T1" if jt <= 3 else "T2"
    return "T1" if jt <= 7 else "T2"


# pt slot order: A/B j-tile groups interleaved so exp pairs hit adjacent slots
_SLOT_JTS = {
    0: [0, 1, 2, 3],
    1: [0, 4, 1, 5, 2, 6, 3, 7, 8, 9, 10, 11, 12],
    2: [0, 4, 1, 5, 2, 6, 3, 7, 8, 9, 10, 11, 12],
}


def _unit_slots(g):
    """Per pt-slot: (jt, slot, a, main_cl, tail_cl)."""
    out = []
    for slot, jt in enumerate(_SLOT_JTS[g]):
        a, ln = _grp_interval(g, jt)
        out.append((jt, slot, a, min(ln, 512), max(0, ln - 512)))
    return out


def _build_program():
    nc = bacc.Bacc("TRN2", target_bir_lowering=False, debug=False)

    xT_d = nc.dram_tensor("xT", [C, L], F16, kind="ExternalInput")
    wq_d = nc.dram_tensor("wqT", [C, G], F16, kind="ExternalInput")
    wk_d = nc.dram_tensor("wkT", [C, G], F16, kind="ExternalInput")
    wv_d = nc.dram_tensor("wvT", [C, G], F16, kind="ExternalInput")
    wp_d = nc.dram_tensor("wpT", [G, C], F16, kind="ExternalInput")
    bq_d = nc.dram_tensor("bqP", [P, CT], F32, kind="ExternalInput")
    bk_d = nc.dram_tensor("bkP", [P, CT], F32, kind="ExternalInput")
    mstk_d = nc.dram_tensor("maskStk", [P, 2 * P], F16, kind="ExternalInput")
    maskt_d = nc.dram_tensor("maskTxt", [32, 1024], F16, kind="ExternalInput")
    out_d = nc.dram_tensor("out_part", [L, C], F16, kind="ExternalOutput")

    units = [(g, h) for g in range(3) for h in range(NH)]

    with tile.TileContext(nc) as tc, ExitStack() as big:
        persist = big.enter_context(tc.tile_pool(name="persist", bufs=1))
        phA = big.enter_context(tc.tile_pool(name="phA", bufs=1))
        phB = big.enter_context(tc.tile_pool(name="phB", bufs=1))
        psPair = big.enter_context(tc.tile_pool(name="psPair", bufs=2, space="PSUM"))
        psYp = big.enter_context(tc.tile_pool(name="psYp", bufs=2, space="PSUM"))
        psMix = big.enter_context(tc.tile_pool(name="psMix", bufs=2, space="PSUM"))

        # persistent SBUF tensors
        qT = persist.tile([P, CT, L], F16, name="qT")
        kT = persist.tile([P, CT, L], F16, name="kT")
        v_ones = persist.tile([P, NJT, NH, VW], F16, name="v_ones")
        maskStk = persist.tile([P, 2, P], F16, name="maskStk_sb")
        maskTx = persist.tile([32, 1024], F16, name="maskTx_sb")
        yT = persist.tile([P, CT, L], F16, name="yT")
        wp_sb = persist.tile([P, CT, C], F16, name="wp_sb")
        warm = persist.tile([P, 512], F16, name="warm")

        # small memset first: PE warmup dummies depend only on it
        nc.gpsimd.memset(warm[:], 0.125)
        # ones column at free index 0 (-> den at PSUM partition 0); cols
        # 1..63 stay 1.0 but psY rows 1..63 are never read.
        nc.gpsimd.memset(v_ones[:], 1.0)

        # ---------- input tiles + DMA, critical-path order ----------
        # First the 12 tiles the very first k/q chains need (wk, x-ich0),
        # then wq/biases/wv, then the big non-critical loads (maskStk, wp)
        # split into chunks so no queue is hogged for 30us.
        xT = phA.tile([P, ET, L], F16, name="xT_sb")
        wq_sb = phA.tile([P, ET, G], F16, name="wq_sb")
        wk_sb = phA.tile([P, ET, G], F16, name="wk_sb")
        wv_sb = phA.tile([P, ET, G], F16, name="wv_sb")
        bq_sb = phA.tile([P, CT], F32, name="bq_sb")
        bk_sb = phA.tile([P, CT], F32, name="bk_sb")

        for et in range(ET):
            nc.sync.dma_start(wk_sb[:, et, :], wk_d[et * P : (et + 1) * P, :])
            nc.sync.dma_start(xT[:, et, 0:512], xT_d[et * P : (et + 1) * P, 0:512])
        nc.sync.dma_start(bk_sb[:], bk_d[:])
        nc.sync.dma_start(bq_sb[:], bq_d[:])
        for et in range(ET):
            nc.sync.dma_start(wq_sb[:, et, :], wq_d[et * P : (et + 1) * P, :])
        for et in range(ET):
            nc.sync.dma_start(wv_sb[:, et, :], wv_d[et * P : (et + 1) * P, :])
        nc.sync.dma_start(maskStk[:], mstk_d.rearrange("p (s c) -> p s c", c=P))
        nc.sync.dma_start(maskTx[:], maskt_d[:])
        for i0, ilen in I_CHUNKS[1:]:
            for et in range(ET):
                nc.sync.dma_start(
                    xT[:, et, i0 : i0 + ilen],
                    xT_d[et * P : (et + 1) * P, i0 : i0 + ilen],
                )
        wp_v = wp_d.rearrange("(ct p) n -> p ct n", p=P)
        for ct in range(CT):
            nc.sync.dma_start(wp_sb[:, ct, :], wp_v[:, ct, :])

        # ---------- PE p-state warmup (no DMA deps) ----------
        for d in range(10):
            dt_ = psPair.tile([P, 2, 512], F32, name="ps_warm", tag="ps_pair")
            nc.tensor.matmul(
                dt_[:, d % 2, :],
                warm[:, 0:P],
                warm[:, :],
                start=True,
                stop=True,
                skip_group_check=True,
            )

        # ---------- projection chain emitters ----------
        def emit_qk_chain(dst, w_sb, b_sb, ct, ich):
            i0, ilen = I_CHUNKS[ich]
            ps = psMix.tile([P, 512], F32, name="ps_p", tag="ps_mix")
            for et in range(ET):
                nc.tensor.matmul(
                    ps[:, :ilen],
                    w_sb[:, et, ct * P : (ct + 1) * P],
                    xT[:, et, i0 : i0 + ilen],
                    start=(et == 0),
                    stop=(et == ET - 1),
                    skip_group_check=True,
                )
            nc.vector.tensor_scalar(
                dst[:, ct, i0 : i0 + ilen],
                ps[:, :ilen],
                b_sb[:, ct : ct + 1],
                None,
                mybir.AluOpType.add,
            )

        def emit_v_chain(it):
            il = _jl(it)
            ps = psMix.tile([P, 512], F32, name="ps_pv", tag="ps_mix")
            for et in range(ET):
                nc.tensor.matmul(
                    ps[:il, :G],
                    xT[:, et, it * P : it * P + il],
                    wv_sb[:, et, :],
                    start=(et == 0),
                    stop=(et == ET - 1),
                    skip_group_check=True,
                )
            nc.vector.tensor_copy(
                v_ones[:il, it, :, V0 : V0 + D],
                ps[:il, :G].rearrange("p (h d) -> p h d", h=NH),
            )

        def emit_outproj_chain(it, nch, tail=False):
            il = _jl(it)
            ps_o = psMix.tile([P, 512], F32, name="ps_po", tag="ps_mix")
            for kt in range(CT):
                nc.tensor.matmul(
                    ps_o[:il, :G],
                    yT[:, kt, it * P : it * P + il],
                    wp_sb[:, kt, nch * G : (nch + 1) * G],
                    start=(kt == 0),
                    stop=(kt == CT - 1),
                    skip_group_check=True,
                )
            o_sb = phB.tile([P, G], F16, name="o_sb", tag="o_sb", bufs=6)
            if tail and (it + nch) % 2 == 0:
                # ACT is idle in the drain tail - halve the cast serialization
                nc.scalar.copy(o_sb[:il, :], ps_o[:il, :G])
            else:
                nc.vector.tensor_copy(o_sb[:il, :], ps_o[:il, :G])
            r0 = it * P
            c0 = nch * G
            if tail and il == P:
                # split across queues + issue from the idle ACT sequencer so
                # the sync queue can run its end-of-program drain
                nc.scalar.dma_start(out_d[r0 : r0 + 64, c0 : c0 + G], o_sb[0:64, :])
                nc.scalar.dma_start(out_d[r0 + 64 : r0 + il, c0 : c0 + G],
                                    o_sb[64:il, :])
            else:
                nc.sync.dma_start(out_d[r0 : r0 + il, c0 : c0 + G], o_sb[:il, :])

        # ---------- attention emitters ----------
        urec = [dict() for _ in units]

        def kq(i, jt, ca, cl):
            g, h = units[i]
            pof = D * (h % 2)
            ct = h // 2
            jl = _jl(jt)
            return (
                kT[pof : pof + D, ct, jt * P : jt * P + jl],
                qT[pof : pof + D, ct, ca : ca + cl],
            )

        def emit_pair(i, s1, s2):
            # two equal-ish chunks -> one 2-bank psum tile -> ONE exp
            jt1, slot1, a1, cl1, _ = s1
            jt2, slot2, a2, cl2, _ = s2
            pt = urec[i]["pt"]
            tile_ = psPair.tile([P, 2, 512], F32, name="ps_s2", tag="ps_pair")
            for idx, (jt, ca, cl) in enumerate(((jt1, a1, cl1), (jt2, a2, cl2))):
                k_ap, q_ap = kq(i, jt, ca, cl)
                nc.tensor.matmul(
                    tile_[: _jl(jt), idx, :cl], k_ap, q_ap,
                    start=True, stop=True, skip_group_check=True,
                )
            W = max(cl1, cl2)
            nc.scalar.activation(
                pt[:P, slot1 : slot1 + 2, 0:W],
                tile_[:P, :, 0:W],
                mybir.ActivationFunctionType.Exp,
                bias=0.0,
                scale=SCALE,
            )

        def emit_single12(i, s):
            jt, slot, a, mcl, tcl = s
            pt = urec[i]["pt"]
            jl = _jl(jt)  # 32
            tile_ = psPair.tile([P, 2, 512], F32, name="ps_s1", tag="ps_pair")
            k_ap, q_ap = kq(i, jt, a, mcl)
            nc.tensor.matmul(
                tile_[:jl, 0, :mcl], k_ap, q_ap,
                start=True, stop=True, skip_group_check=True,
            )
            if tcl:
                k_ap, q_ap = kq(i, jt, a + 512, tcl)
                nc.tensor.matmul(
                    tile_[:jl, 1, :tcl], k_ap, q_ap,
                    start=True, stop=True, skip_group_check=True,
                )
            flat = tile_[:jl].rearrange("p a b -> p (a b)")
            nc.scalar.activation(
                pt[:jl, slot, 0 : mcl + tcl],
                flat[:, 0 : mcl + tcl],
                mybir.ActivationFunctionType.Exp,
                bias=0.0,
                scale=SCALE,
            )

        def emit_tailpair(i, s1, s2):
            # the 32-col score tails of two adjacent 544-wide slots
            pt = urec[i]["pt"]
            mix = psMix.tile([P, 512], F32, name="ps_st", tag="ps_mix")
            for idx, (jt, slot, a, mcl, tcl) in enumerate((s1, s2)):
                k_ap, q_ap = kq(i, jt, a + 512, tcl)
                nc.tensor.matmul(
                    mix[: _jl(jt), idx * 32 : idx * 32 + 32], k_ap, q_ap,
                    start=True, stop=True, skip_group_check=True,
                )
            nc.scalar.activation(
                pt[:P, s1[1] : s1[1] + 2, 512:544],
                mix[:P, 0:64].rearrange("p (a b) -> p a b", b=32),
                mybir.ActivationFunctionType.Exp,
                bias=0.0,
                scale=SCALE,
            )

        def emit_tail8(i, s):
            jt, slot, a, mcl, tcl = s
            pt = urec[i]["pt"]
            mix = psMix.tile([P, 512], F32, name="ps_st8", tag="ps_mix")
            k_ap, q_ap = kq(i, jt, a + 512, tcl)
            nc.tensor.matmul(
                mix[: _jl(jt), 0:32], k_ap, q_ap,
                start=True, stop=True, skip_group_check=True,
            )
            nc.scalar.activation(
                pt[:P, slot, 512:544],
                mix[:P, 0:32],
                mybir.ActivationFunctionType.Exp,
                bias=0.0,
                scale=SCALE,
            )

        def emit_mask_window(i, w):
            # maskStk holds just the two distinct diag masks (T1 at 0, T2 at
            # 1); window patterns are XXXX or XYXY -> broadcast APs.
            # (GpSimd is too slow for these: ~1.1us/window + microcode
            # library reloads against PartitionBroadcast, and the windows
            # gate attv - keep them on DVE.)
            g, h = units[i]
            pt = urec[i]["pt"]
            s0 = 4 * w
            kinds = [_grp_diag(g, jt) for jt in _SLOT_JTS[g][s0 : s0 + 4]]
            dst = pt[:, s0 : s0 + 4, 0:P]
            if kinds[0] != kinds[1]:  # (T1,T2,T1,T2)
                dv = dst.rearrange("p (r t) c -> p r t c", t=2)
                mv = maskStk[:, None, 0:2, :].to_broadcast([P, 2, 2, P])
                nc.vector.tensor_tensor(dv, dv, mv, mybir.AluOpType.mult)
            else:
                idx = 0 if kinds[0] == "T1" else 1
                mv = maskStk[:, idx : idx + 1, :].to_broadcast([P, 4, P])
                nc.vector.tensor_tensor(dst, dst, mv, mybir.AluOpType.mult)

        def emit_text_mask(i):
            g, h = units[i]
            pt = urec[i]["pt"]
            m0 = 0 if g == 1 else 512
            nc.vector.tensor_tensor(
                pt[0:32, 12, 0:512],
                pt[0:32, 12, 0:512],
                maskTx[0:32, m0 : m0 + 512],
                mybir.AluOpType.mult,
            )

        def attv_items(i):
            """Flat list: ("part", ich, jt, slot, pl, poff, off, first, stop)
            and ("drain", ich, is_final) items, ich groups in drain order
            (g2: ich3 first - its psY bank frees mid-block)."""
            g, h = units[i]
            groups = {}
            for jt, slot, ca, cl, soff in urec[i]["chunks"]:
                subs = [(ca, cl, soff)]
                if ca < 1536 < ca + cl:
                    subs = [
                        (ca, 1536 - ca, soff),
                        (1536, ca + cl - 1536, soff + 1536 - ca),
                    ]
                for pa, pl, poff in subs:
                    ich = _ich_of(pa)
                    groups.setdefault(ich, []).append(
                        (jt, slot, pa, pl, poff)
                    )
            ich_order = sorted(groups, key=lambda c: len(groups[c]) * 1000 + c)
            if units[i][0] == 2:
                ich_order = [3, 2]
            items = []
            for gi, ich in enumerate(ich_order):
                parts = groups[ich]
                for pi, (jt, slot, pa, pl, poff) in enumerate(parts):
                    items.append(
                        ("part", ich, jt, slot, pa, pl, poff,
                         pi == 0, pi == len(parts) - 1)
                    )
                items.append(("drain", ich, gi == len(ich_order) - 1))
            return items

        def emit_attv_item(i, item):
            g, h = units[i]
            if item[0] == "part":
                _, ich, jt, slot, pa, pl, poff, first, stop = item
                jl = _jl(jt)
                ysd = urec[i].setdefault("ps_y", {})
                if ich not in ysd:
                    ysd[ich] = psYp.tile([VW, 512], F32, name="ps_y", tag="ps_y")
                off = pa - ICH0[ich]
                nc.tensor.matmul(
                    ysd[ich][:, off : off + pl],
                    v_ones[:jl, jt, h, :],
                    urec[i]["pt"][:jl, slot, poff : poff + pl],
                    start=first,
                    stop=stop,
                    skip_group_check=True,
                )
            else:
                _, ich, is_final = item
                psy = urec[i]["ps_y"][ich]
                ilen = I_CHUNKS[ich][1]
                rc = phB.tile([1, 512], F32, name="rc", tag="rc", bufs=4)
                nc.vector.reciprocal_approx_fast(
                    out=rc[0:1, :ilen], in_=psy[0:1, :ilen]
                )
                rc_bc = phB.tile([D, 512], F32, name="rc_bc", tag="rc_bc", bufs=4)
                nc.gpsimd.partition_broadcast(rc_bc[:, :ilen], rc[0:1, :ilen])
                if is_final:
                    urec[i]["pending"] = (ich, psy, rc_bc)
                else:
                    emit_norm(i, ich, psy, rc_bc)

        def emit_norm(i, ich, psy, rc_bc):
            g, h = units[i]
            pof = D * (h % 2)
            ct = h // 2
            i0, ilen = I_CHUNKS[ich]
            nc.vector.tensor_tensor(
                yT[pof : pof + D, ct, i0 : i0 + ilen],
                psy[V0 : V0 + D, :ilen],
                rc_bc[:, :ilen],
                mybir.AluOpType.mult,
            )

        def emit_pending_norm(i):
            if "pending" in urec[i]:
                ich, psy, rc_bc = urec[i].pop("pending")
                emit_norm(i, ich, psy, rc_bc)

        # ---------- filler segments ----------
        f1 = deque()
        for ct in range(CT):
            if ct == 0:
                f1.append(lambda: emit_qk_chain(qT, wq_sb, bq_sb, 0, 1))
                for ich in (1, 2, 3):
                    f1.append(
                        lambda ich=ich: emit_qk_chain(kT, wk_sb, bk_sb, 0, ich)
                    )
                for it in range(4, NJT):
                    f1.append(lambda it=it: emit_v_chain(it))
            else:
                f1.append(lambda ct=ct: emit_qk_chain(qT, wq_sb, bq_sb, ct, 1))
                for ich in (1, 2, 3):
                    f1.append(
                        lambda ct=ct, ich=ich: emit_qk_chain(
                            kT, wk_sb, bk_sb, ct, ich
                        )
                    )
        f2 = deque()
        for ich in (2, 3):
            for ct in range(CT):
                f2.append(
                    lambda ct=ct, ich=ich: emit_qk_chain(qT, wq_sb, bq_sb, ct, ich)
                )
        f3 = deque(
            (lambda it=it, nch=nch: emit_outproj_chain(it, nch))
            for it in range(0, 4)
            for nch in range(2)
        )
        f4 = deque(
            (lambda it=it, nch=nch: emit_outproj_chain(it, nch))
            for it in range(4, 8)
            for nch in range(2)
        )
        # explicit per-block filler schedule: keep late blocks supplied so
        # the PE never outruns ACT and drops p-state
        SCHED = [[] for _ in range(len(units))]
        for blk, cnt in zip(range(0, 6), (4, 4, 4, 3, 3, 3)):
            for _ in range(cnt):
                SCHED[blk].append(f1.popleft())
        for blk in range(6, 12):
            SCHED[blk].append(f2.popleft())
        for blk, cnt in zip(range(7, 13), (1, 1, 1, 1, 2, 2)):
            for _ in range(cnt):
                SCHED[blk].append(f3.popleft())
        for blk, cnt in zip(range(13, 18), (2, 2, 2, 1, 1)):
            for _ in range(cnt):
                SCHED[blk].append(f4.popleft())
        assert not (f1 or f2 or f3 or f4)

        # ---------- upfront: phase A for g0's needs ----------
        for ct in range(CT):
            emit_qk_chain(kT, wk_sb, bk_sb, ct, 0)
            emit_qk_chain(qT, wq_sb, bq_sb, ct, 0)
        for it in range(4):
            emit_v_chain(it)

        # ---------- main software-pipelined loop ----------
        for i, (g, h) in enumerate(units):
            if i >= 2:
                emit_pending_norm(i - 2)
            urec[i]["pt"] = phB.tile(
                [P, NJT, SLOT], F16, name="pt", tag="pt", bufs=3
            )
            slots = _unit_slots(g)
            chunks = []
            for jt, slot, a, mcl, tcl in slots:
                chunks.append((jt, slot, a, mcl, 0))
                if tcl:
                    chunks.append((jt, slot, a + 512, tcl, 512))
            urec[i]["chunks"] = chunks

            # exp-units: (fn, main_slot_done)
            eus = []
            if g == 0:
                eus.append((lambda s=slots: emit_pair(i, s[0], s[1]), 1))
                eus.append((lambda s=slots: emit_pair(i, s[2], s[3]), 3))
            else:
                for k in range(6):
                    eus.append(
                        (lambda s=slots, k=k: emit_pair(i, s[2 * k], s[2 * k + 1]),
                         2 * k + 1)
                    )
                eus.append((lambda s=slots: emit_single12(i, s[12]), 12))
                if g == 2:
                    eus.append((lambda s=slots: emit_tailpair(i, s[0], s[1]), -1))
                    eus.append((lambda s=slots: emit_tail8(i, s[8]), -1))

            av = attv_items(i - 1) if i >= 1 else []
            # pace attv by COLUMNS (ich3 parts are tiny 32-col matmuls; by
            # count they starve the first half of the block of PE work)
            avw = [it_[5] if it_[0] == "part" else 0 for it_ in av]
            avtot = max(1, sum(avw))
            fillers = SCHED[i]
            nF = len(fillers)
            nE = len(eus)
            nwin = 1 if g == 0 else 3
            pops = 0
            ai = 0
            acum = 0
            next_w = 0
            for e_idx, (fn, sdone) in enumerate(eus):
                fn()
                while next_w < nwin and 4 * next_w + 3 <= sdone:
                    emit_mask_window(i, next_w)
                    next_w += 1
                if sdone == 12 and g >= 1:
                    emit_text_mask(i)
                tgt = (e_idx + 1) * avtot // nE
                while ai < len(av) and (
                    acum < tgt or av[ai][0] == "drain"
                ):
                    emit_attv_item(i - 1, av[ai])
                    acum += avw[ai]
                    ai += 1
                if pops < nF and (e_idx + 1) * nF >= (pops + 1) * nE:
                    fillers[pops]()
                    pops += 1
            while ai < len(av):
                emit_attv_item(i - 1, av[ai])
                ai += 1
            while pops < nF:
                fillers[pops]()
                pops += 1

        # ---------- tail ----------
        nu = len(units)
        emit_pending_norm(nu - 2)
        av = attv_items(nu - 1)
        for item in av:
            emit_attv_item(nu - 1, item)
            if item[0] == "drain" and item[1] == 3:
                # it12 out-proj needs only the ich3 norms (text rows), which
                # just completed - run it while the ich2 attv still streams
                for nch in range(2):
                    emit_outproj_chain(12, nch, tail=True)
        emit_pending_norm(nu - 1)
        for it in range(8, 12):
            for nch in range(2):
                emit_outproj_chain(it, nch, tail=True)

    nc.compile()
    return nc


def _build_mask_np(seg_starts, seg_ends):
    """True = masked. Mirrors reference._build_mask in numpy."""
    ML = 3 * T
    tril = np.tril(np.ones((T, T), dtype=bool))
    sl = np.tril(np.ones((T, T), dtype=bool), -1)
    m = np.zeros((L, L), dtype=bool)
    m[:ML, :ML] = True
    m[0:T, 0:T] = ~tril
    m[T : 2 * T, 0:T] = ~tril
    m[T : 2 * T, T : 2 * T] = ~sl
    m[T : 2 * T, 2 * T : 3 * T] = ~sl
    m[2 * T : 3 * T, 0:T] = ~tril
    m[2 * T : 3 * T, T : 2 * T] = ~tril
    m[2 * T : 3 * T, 2 * T : 3 * T] = ~sl
    m[:ML, ML:] = True
    frames = np.arange(T)[None, :, None]
    allowed = (frames >= seg_starts[:, None, :]) & (frames < seg_ends[:, None, :])
    mask = np.broadcast_to(m[None], (B, L, L)).copy()
    for row0, col_blocks in ((T, (0, 2, 3)), (2 * T, (1, 2, 3))):
        for j in col_blocks:
            c0 = ML + j * N
            mask[:, row0 : row0 + T, c0 : c0 + N] &= ~allowed
    return mask


def get_nc():
    global _NC
    if _NC is None:
        _NC = _build_program()
    return _NC


def _build_maskstk():
    r = np.arange(P)
    t1 = (r[:, None] <= r[None, :]).astype(np.float16)  # tril.T
    t2 = (r[:, None] < r[None, :]).astype(np.float16)  # strict
    stk = np.empty((P, 2, P), dtype=np.float16)
    stk[:, 0, :] = t1
    stk[:, 1, :] = t2
    return stk.reshape(P, 2 * P)


def make_in_maps(x, Wq, bq, Wk, bk, Wv, bv, Wp, bp, seg_starts, seg_ends):
    mask = _build_mask_np(np.asarray(seg_starts), np.asarray(seg_ends))
    maskstk = _build_maskstk()
    in_maps = []
    for core in range(8):
        b, g = core // 2, core % 2
        gs = slice(g * G, (g + 1) * G)
        allowT = ~mask[b].T  # [j, i]
        maskTx = np.ascontiguousarray(
            allowT[1536:1568, 512:1536].astype(np.float16)
        )
        in_maps.append(
            {
                "xT": np.ascontiguousarray(x[b].T).astype(np.float16),
                "wqT": np.ascontiguousarray(Wq[gs, :].T).astype(np.float16),
                "wkT": np.ascontiguousarray(Wk[gs, :].T).astype(np.float16),
                "wvT": np.ascontiguousarray(Wv[gs, :].T).astype(np.float16),
                "wpT": np.ascontiguousarray(Wp[:, gs].T).astype(np.float16),
                "bqP": np.ascontiguousarray(bq[gs].reshape(CT, P).T),
                "bkP": np.ascontiguousarray(bk[gs].reshape(CT, P).T),
                "maskStk": maskstk,
                "maskTxt": maskTx,
            }
        )
    return in_maps


def kernel(x, Wq, bq, Wk, bk, Wv, bv, Wp, bp, seg_starts, seg_ends, T_motion=None,
           N=None, _trace=False, **_unused):
    x = np.asarray(x, np.float32)
    args = [np.asarray(a, np.float32) for a in (Wq, bq, Wk, bk, Wv, bv, Wp, bp)]
    Wq, bq, Wk, bk, Wv, bv, Wp, bp = args
    nc = get_nc()
    in_maps = make_in_maps(x, Wq, bq, Wk, bk, Wv, bv, Wp, bp, seg_starts, seg_ends)
    res = run_bass_kernel_spmd(nc, in_maps, core_ids=list(range(8)), trace=_trace)
    parts = [np.asarray(r["out_part"], np.float32) for r in res.results]
    # v-bias folds into the output bias exactly: att rows sum to 1, so
    # y = att@(v+bv) = att@v + bv, and (y+bv)@Wp.T = y@Wp.T + bv@Wp.T
    bp_eff = bp + bv @ Wp.T
    y = np.empty((B, L, C), np.float32)
    for b in range(B):
        y[b] = parts[2 * b] + parts[2 * b + 1] + bp_eff
    if _trace:
        kernel.last_results = res
    return y
